# revision 3
# baseline (speedup 1.0000x reference)
"""Trainium2 Bass kernel for nn_DLCF_DCA (scatter_memory).

Reference computation, per sample b (B=128, S=256, H=768, K=64):
  keep_dep[s]  = (s==0) or any_k(depend[b,k] == s-1)
  keep_dpd[s]  = (s==0) or any_k(depended[b,k] == s-1)
  mult[s]      = w2 if s-1 in depended else (w1 if s-1 in depend else 0);
                 0 if s-1 in no_connect; 1 if s==0
  y1 = x * keep_dep;  y2 = x * keep_dpd;  y3 = x * mult

Strategy: pure data parallel over batch (16 samples per core, 8 cores).
On device, per-position multipliers are built via one-hot equality compares
(iota vs index lists) reduced over K with tiny tensor-engine matmuls so the
multipliers land with S on the partition dim. The bulk work is then a
streamed read of x in [128, 3072] tiles with three per-partition-scalar
multiplies (vector + scalar engines) and three streamed writes.
"""

import contextlib
import os
import sys

import numpy as np

if "/opt/trn_rl_repo" not in sys.path:
    sys.path.insert(0, "/opt/trn_rl_repo")

N_CORES = 8
B, S, H, K = 128, 256, 768, 64
BL = B // N_CORES          # samples per core
NP = BL * S // 128         # 32 partition groups of 128 token-rows
G = 4                      # partition groups per DMA (1.5 MiB transfers)
ND = NP // G               # DMA iterations
NPAIR = BL // 2            # sample pairs (2 samples x 64 idx = 128 partitions)

_cache = {}


def _split_multiwaits(nc, max_waits=1):
    """walrus in this container only accepts one sync-wait per instruction;
    splice extra waits onto single-wait NoOps just before the offender."""
    from concourse import mybir

    n = 0
    for func in nc.m.functions:
        for bb in func.blocks:
            insts = bb.instructions
            i = 0
            while i < len(insts):
                ins = insts[i]
                si = getattr(ins, "sync_info", None)
                if si is None or len(si.on_wait) <= max_waits:
                    i += 1
                    continue
                waits = list(si.on_wait)
                keep = waits[-max_waits:]
                extra = waits[:-max_waits]
                nops = []
                for j in range(0, len(extra), max_waits):
                    n += 1
                    nops.append(
                        mybir.InstNoOp(
                            name=f"{ins.name}-ws{n}",
                            sync_info=mybir.SyncInfo(
                                on_wait=extra[j : j + max_waits], on_update=[]
                            ),
                            bass_nofuse=True,
                            engine=ins.engine,
                            ins=[],
                            outs=[],
                        )
                    )
                si.on_wait = keep
                for k, nop in enumerate(nops):
                    insts.insert(i + k, nop)
                i += len(nops) + 1
    return n


def _col(t, h, j):
    # mask column for sample b=2t+j, s-half h within a 32-col block
    return t * 4 + h * 2 + j


def _build():
    import concourse.bass as bass
    import concourse.tile as tile
    from concourse import mybir

    f32 = mybir.dt.float32
    i32 = mybir.dt.int32
    nc = bass.Bass()

    x = nc.dram_tensor("x", [BL * S, H], f32, kind="ExternalInput")
    idx = {
        name: nc.dram_tensor(name, [BL * K], f32, kind="ExternalInput")
        for name in ("dep", "dpd", "noc")
    }
    # wcols[0:32] = depend_weight per (t,h,j) col; wcols[32:64] = depended_weight
    wcols = nc.dram_tensor("wcols", [64], f32, kind="ExternalInput")
    # wsel[k + 64*j, j2] = 1 if j == j2 else 0  (block reduction over K)
    wsel = nc.dram_tensor("wsel", [128, 2], f32, kind="ExternalInput")
    ys = [
        nc.dram_tensor(f"y{i}", [BL * S, H], f32, kind="ExternalOutput")
        for i in (1, 2, 3)
    ]

    with tile.TileContext(nc) as tc, contextlib.ExitStack() as ctx:
        const = ctx.enter_context(tc.tile_pool(name="const", bufs=1))
        epool = ctx.enter_context(tc.tile_pool(name="epool", bufs=2))
        psum = ctx.enter_context(tc.tile_pool(name="psum", bufs=1, space="PSUM"))
        xpool = ctx.enter_context(tc.tile_pool(name="xpool", bufs=3))
        ypool = ctx.enter_context(tc.tile_pool(name="ypool", bufs=3))

        # --- constants / small loads -----------------------------------
        iotai = const.tile([128, S], i32)
        nc.gpsimd.iota(iotai[:], pattern=[[1, S]], base=-1, channel_multiplier=0)
        iotaf = const.tile([128, S], f32)
        nc.vector.tensor_copy(iotaf[:], iotai[:])  # cast to f32 (-1..254)

        idxT = {}
        for name, dram in idx.items():
            t = const.tile([128, NPAIR], f32, name=f"idx_{name}")
            nc.sync.dma_start(out=t[:], in_=dram.rearrange("(t p) -> p t", p=128))
            idxT[name] = t
        wselt = const.tile([128, 2], f32)
        nc.sync.dma_start(out=wselt[:], in_=wsel[:, :])
        wrow = const.tile([1, 64], f32)
        nc.sync.dma_start(out=wrow[:], in_=wcols.rearrange("(p n) -> p n", p=1))
        ones1 = const.tile([1, 128], f32)
        nc.vector.memset(ones1[:], 1.0)

        # --- membership counts: C[s_local, col] ------------------------
        # col = L*32 + t*4 + h*2 + j ; value = #{k: idx[L][2t+j, k] == s-1}
        pc = psum.tile([128, 96], f32)
        pw = psum.tile([128, 64], f32)
        nc.tensor.matmul(pw[:], lhsT=ones1[:], rhs=wrow[:], start=True, stop=True)
        wb = const.tile([128, 64], f32)  # W1 | W2 broadcast along partitions
        nc.vector.tensor_copy(wb[:], pw[:])

        for li, name in enumerate(("dep", "dpd", "noc")):
            for t in range(NPAIR):
                e = epool.tile([128, S], f32, name="E")
                nc.vector.tensor_scalar(
                    e[:], iotaf[:], idxT[name][:, t : t + 1], None,
                    op0=mybir.AluOpType.is_equal,
                )
                for h in range(2):
                    c0 = li * 32 + _col(t, h, 0)
                    nc.tensor.matmul(
                        pc[:, c0 : c0 + 2],
                        lhsT=e[:, h * 128 : (h + 1) * 128],
                        rhs=wselt[:],
                        start=True, stop=True,
                    )

        cmask = const.tile([128, 96], f32)
        nc.vector.tensor_copy(cmask[:], pc[:])
        # clamp duplicate-index counts to 1
        nc.vector.tensor_scalar(
            cmask[:], cmask[:], 1.0, None, op0=mybir.AluOpType.min
        )
        # s==0 (partition 0, h==0 cols) keeps the token for m1/m2
        nc.vector.memset(
            cmask[0:1, 0:64].rearrange("p (l t hj) -> p l t hj", l=2, hj=4)[:, :, :, 0:2],
            1.0,
        )

        # m3 = w1 where dep, overwritten by w2 where dpd, zeroed where noc:
        # m3 = (dep1*w1*(1-dpd1) + dpd1*w2) * (1-noc1)
        m3b = const.tile([128, 32], f32)
        tmp = const.tile([128, 32], f32)
        inv = const.tile([128, 32], f32)
        nc.vector.tensor_mul(m3b[:], cmask[:, 0:32], wb[:, 0:32])
        nc.vector.tensor_scalar(  # inv = 1 - dpd1
            inv[:], cmask[:, 32:64], -1.0, 1.0,
            op0=mybir.AluOpType.mult, op1=mybir.AluOpType.add,
        )
        nc.vector.tensor_mul(m3b[:], m3b[:], inv[:])
        nc.vector.tensor_mul(tmp[:], cmask[:, 32:64], wb[:, 32:64])
        nc.vector.tensor_add(m3b[:], m3b[:], tmp[:])
        nc.vector.tensor_scalar(  # inv = 1 - noc1
            inv[:], cmask[:, 64:96], -1.0, 1.0,
            op0=mybir.AluOpType.mult, op1=mybir.AluOpType.add,
        )
        nc.vector.tensor_mul(m3b[:], m3b[:], inv[:])
        nc.vector.memset(
            m3b[0:1, :].rearrange("p (t hj) -> p t hj", hj=4)[:, :, 0:2], 1.0
        )

        # --- streamed multiply ------------------------------------------
        xr = x.rearrange("(d g p) h -> d p g h", p=128, g=G)
        yr = [y.rearrange("(d g p) h -> d p g h", p=128, g=G) for y in ys]
        for d in range(ND):
            xt = xpool.tile([128, G * H], f32, name="xt")
            nc.sync.dma_start(out=xt[:].rearrange("p (g h) -> p g h", g=G), in_=xr[d])
            yt = [ypool.tile([128, G * H], f32, name=f"y{i}t") for i in (1, 2, 3)]
            for g in range(G):
                gg = d * G + g
                b, h = gg // 2, gg % 2
                blk = slice(g * H, (g + 1) * H)
                cdep = 0 + _col(b // 2, h, b % 2)
                cdpd = 32 + _col(b // 2, h, b % 2)
                cm3 = _col(b // 2, h, b % 2)
                nc.vector.tensor_scalar(
                    yt[0][:, blk], xt[:, blk], cmask[:, cdep : cdep + 1], None,
                    op0=mybir.AluOpType.mult,
                )
                nc.vector.tensor_scalar(
                    yt[1][:, blk], xt[:, blk], cmask[:, cdpd : cdpd + 1], None,
                    op0=mybir.AluOpType.mult,
                )
                nc.scalar.activation(
                    yt[2][:, blk], xt[:, blk],
                    mybir.ActivationFunctionType.Copy,
                    scale=m3b[:, cm3 : cm3 + 1],
                )
            for i in range(3):
                nc.sync.dma_start(
                    out=yr[i][d], in_=yt[i][:].rearrange("p (g h) -> p g h", g=G)
                )

    _split_multiwaits(nc)
    return nc


def _prep_inputs(bert_local_out, depend, depended, no_connect,
                 depend_weight, depended_weight):
    x = np.ascontiguousarray(np.asarray(bert_local_out, dtype=np.float32))
    dep = np.asarray(depend).astype(np.float32)
    dpd = np.asarray(depended).astype(np.float32)
    noc = np.asarray(no_connect).astype(np.float32)
    w1 = np.asarray(depend_weight, dtype=np.float32)
    w2 = np.asarray(depended_weight, dtype=np.float32)

    wsel = np.zeros((128, 2), dtype=np.float32)
    wsel[0:64, 0] = 1.0
    wsel[64:128, 1] = 1.0

    in_maps = []
    for c in range(N_CORES):
        sl = slice(c * BL, (c + 1) * BL)
        wcols = np.zeros(64, dtype=np.float32)
        for t in range(NPAIR):
            for h in range(2):
                for j in range(2):
                    col = _col(t, h, j)
                    wcols[col] = w1[c * BL + 2 * t + j]
                    wcols[32 + col] = w2[c * BL + 2 * t + j]
        in_maps.append(
            {
                "x": x[sl].reshape(BL * S, H),
                "dep": dep[sl].reshape(-1),
                "dpd": dpd[sl].reshape(-1),
                "noc": noc[sl].reshape(-1),
                "wcols": wcols,
                "wsel": wsel,
            }
        )
    return in_maps


def kernel(bert_local_out, depend, depended, no_connect,
           depend_weight, depended_weight):
    from concourse.bass_utils import run_bass_kernel_spmd

    if "nc" not in _cache:
        _cache["nc"] = _build()
    nc = _cache["nc"]

    in_maps = _prep_inputs(bert_local_out, depend, depended, no_connect,
                           depend_weight, depended_weight)

    pdir = os.environ.get("KERNEL_PROFILE_DIR")
    ctx = contextlib.nullcontext()
    if pdir:
        import concourse.bass2jax as b2j
        from trn_agent_boot.trn_boot import _ntff_profile_via_ctypes

        if not getattr(b2j, "_neff_capture_patched", False):
            orig = b2j.rename_neff_tensors_and_patch_header

            def patched(neff_path, mapping):
                data = orig(neff_path, mapping)
                cap = os.environ.get("KERNEL_PROFILE_DIR")
                if cap:
                    os.makedirs(cap, exist_ok=True)
                    with open(os.path.join(cap, "model.neff"), "wb") as f:
                        f.write(data)
                return data

            b2j.rename_neff_tensors_and_patch_header = patched
            b2j._neff_capture_patched = True
        os.makedirs(pdir, exist_ok=True)
        hookf = _ntff_profile_via_ctypes("/opt/axon/libaxon_pjrt.so")
        if hookf is not None:
            ctx = hookf(pdir, [0])

    with ctx:
        res = run_bass_kernel_spmd(nc, in_maps, list(range(N_CORES)))

    outs = []
    for name in ("y1", "y2", "y3"):
        full = np.empty((B, S, H), dtype=np.float32)
        for c in range(N_CORES):
            full[c * BL : (c + 1) * BL] = res.results[c][name].reshape(BL, S, H)
        outs.append(full)
    return tuple(outs)


# revision 4
# speedup vs baseline: 1.1301x; 1.1301x over previous
"""Trainium2 Bass kernel for nn_DLCF_DCA (scatter_memory).

Reference computation, per sample b (B=128, S=256, H=768, K=64):
  keep_dep[s]  = (s==0) or any_k(depend[b,k] == s-1)
  keep_dpd[s]  = (s==0) or any_k(depended[b,k] == s-1)
  mult[s]      = w2 if s-1 in depended else (w1 if s-1 in depend else 0);
                 0 if s-1 in no_connect; 1 if s==0
  y1 = x * keep_dep;  y2 = x * keep_dpd;  y3 = x * mult

Strategy: pure data parallel over batch (16 samples per core, 8 cores).
Each core streams its [4096, 768] shard with 32 consecutive token-rows per
SBUF partition so every DMA moves long contiguous chunks per partition.
Multiplier masks are built in the matching [partition, row-in-partition]
layout: each index k is decomposed as (q, r) = divmod(b*256 + idx + 1, 32)
and membership counts come from one-hot compares contracted on the tensor
engine (count[p, r] = sum_k Q[k,p] * R[k,r]). The bulk work is then three
per-partition-scalar multiplies per 768-wide row block (vector + scalar
engines) between streamed input and output DMAs.
"""

import contextlib
import os
import sys

import numpy as np

if "/opt/trn_rl_repo" not in sys.path:
    sys.path.insert(0, "/opt/trn_rl_repo")

N_CORES = 8
B, S, H, K = 128, 256, 768, 64
BL = B // N_CORES          # samples per core
ROWS = BL * S              # 4096 token-rows per core
RPP = ROWS // 128          # 32 consecutive rows per partition
ND = 8                     # DMA tiles over the free dim
RPT = RPP // ND            # 4 row-blocks per tile
NCHUNK = BL * K // 128     # 8 contraction chunks for membership counts

_cache = {}


def _split_multiwaits(nc, max_waits=1):
    """walrus in this container only accepts one sync-wait per instruction;
    splice extra waits onto single-wait NoOps just before the offender."""
    from concourse import mybir

    n = 0
    for func in nc.m.functions:
        for bb in func.blocks:
            insts = bb.instructions
            i = 0
            while i < len(insts):
                ins = insts[i]
                si = getattr(ins, "sync_info", None)
                if si is None or len(si.on_wait) <= max_waits:
                    i += 1
                    continue
                waits = list(si.on_wait)
                keep = waits[-max_waits:]
                extra = waits[:-max_waits]
                nops = []
                for j in range(0, len(extra), max_waits):
                    n += 1
                    nops.append(
                        mybir.InstNoOp(
                            name=f"{ins.name}-ws{n}",
                            sync_info=mybir.SyncInfo(
                                on_wait=extra[j : j + max_waits], on_update=[]
                            ),
                            bass_nofuse=True,
                            engine=ins.engine,
                            ins=[],
                            outs=[],
                        )
                    )
                si.on_wait = keep
                for k, nop in enumerate(nops):
                    insts.insert(i + k, nop)
                i += len(nops) + 1
    return n


def _build():
    import concourse.bass as bass
    import concourse.tile as tile
    from concourse import mybir

    f32 = mybir.dt.float32
    i32 = mybir.dt.int32
    eq = mybir.AluOpType.is_equal
    nc = bass.Bass()

    x = nc.dram_tensor("x", [ROWS, H], f32, kind="ExternalInput")
    # per list: q[entry], r[entry] with entry = (b, k) flattened, padded p0 list
    qv = {n: nc.dram_tensor(f"q_{n}", [BL * K], f32, kind="ExternalInput")
          for n in ("dep", "dpd", "noc")}
    rv = {n: nc.dram_tensor(f"r_{n}", [BL * K], f32, kind="ExternalInput")
          for n in ("dep", "dpd", "noc")}
    p0q = nc.dram_tensor("p0q", [128], f32, kind="ExternalInput")
    p0r = nc.dram_tensor("p0r", [128], f32, kind="ExternalInput")
    w1p = nc.dram_tensor("w1p", [128], f32, kind="ExternalInput")
    w2p = nc.dram_tensor("w2p", [128], f32, kind="ExternalInput")
    ys = [nc.dram_tensor(f"y{i}", [ROWS, H], f32, kind="ExternalOutput")
          for i in (1, 2, 3)]

    with tile.TileContext(nc) as tc, contextlib.ExitStack() as ctx:
        const = ctx.enter_context(tc.tile_pool(name="const", bufs=1))
        epool = ctx.enter_context(tc.tile_pool(name="epool", bufs=2))
        psum = ctx.enter_context(tc.tile_pool(name="psum", bufs=1, space="PSUM"))
        xpool = ctx.enter_context(tc.tile_pool(name="xpool", bufs=3))
        ypool = ctx.enter_context(tc.tile_pool(name="ypool", bufs=3))

        # --- small loads / iota -----------------------------------------
        qT, rT = {}, {}
        for n in ("dep", "dpd", "noc"):
            tq = const.tile([128, NCHUNK], f32, name=f"q_{n}")
            nc.sync.dma_start(out=tq[:], in_=qv[n].rearrange("(c p) -> p c", p=128))
            tr = const.tile([128, NCHUNK], f32, name=f"r_{n}")
            nc.sync.dma_start(out=tr[:], in_=rv[n].rearrange("(c p) -> p c", p=128))
            qT[n], rT[n] = tq, tr
        p0qT = const.tile([128, 1], f32)
        nc.sync.dma_start(out=p0qT[:], in_=p0q.rearrange("(p c) -> p c", c=1))
        p0rT = const.tile([128, 1], f32)
        nc.sync.dma_start(out=p0rT[:], in_=p0r.rearrange("(p c) -> p c", c=1))
        w1T = const.tile([128, 1], f32)
        nc.sync.dma_start(out=w1T[:], in_=w1p.rearrange("(p c) -> p c", c=1))
        w2T = const.tile([128, 1], f32)
        nc.sync.dma_start(out=w2T[:], in_=w2p.rearrange("(p c) -> p c", c=1))

        iotai = const.tile([128, 128], i32)
        nc.gpsimd.iota(iotai[:], pattern=[[1, 128]], base=0, channel_multiplier=0)
        iota = const.tile([128, 128], f32)
        nc.vector.tensor_copy(iota[:], iotai[:])  # 0..127 along free dim

        # --- membership counts: pc[p, L*32 + r] --------------------------
        pc = psum.tile([128, 128], f32)
        for li, n in enumerate(("dep", "dpd", "noc")):
            for c in range(NCHUNK):
                qt = epool.tile([128, 128], f32, name="Q")
                nc.vector.tensor_scalar(qt[:], iota[:], qT[n][:, c : c + 1], None, op0=eq)
                rt = epool.tile([128, RPP], f32, name="R")
                nc.vector.tensor_scalar(rt[:], iota[:, 0:RPP], rT[n][:, c : c + 1], None, op0=eq)
                nc.tensor.matmul(
                    pc[:, li * 32 : li * 32 + 32], lhsT=qt[:], rhs=rt[:],
                    start=(c == 0), stop=(c == NCHUNK - 1),
                )
        qt = epool.tile([128, 128], f32, name="Q")
        nc.vector.tensor_scalar(qt[:], iota[:], p0qT[:], None, op0=eq)
        rt = epool.tile([128, RPP], f32, name="R")
        nc.vector.tensor_scalar(rt[:], iota[:, 0:RPP], p0rT[:], None, op0=eq)
        nc.tensor.matmul(pc[:, 96:128], lhsT=qt[:], rhs=rt[:], start=True, stop=True)

        cm = const.tile([128, 128], f32)
        nc.vector.tensor_copy(cm[:], pc[:])
        nc.vector.tensor_scalar(cm[:], cm[:], 1.0, None, op0=mybir.AluOpType.min)
        dep1, dpd1 = cm[:, 0:32], cm[:, 32:64]
        noc1, p0m = cm[:, 64:96], cm[:, 96:128]

        # m1/m2: membership OR position-0
        m1 = const.tile([128, RPP], f32)
        nc.vector.tensor_tensor(m1[:], dep1, p0m, op=mybir.AluOpType.max)
        m2 = const.tile([128, RPP], f32)
        nc.vector.tensor_tensor(m2[:], dpd1, p0m, op=mybir.AluOpType.max)

        # m3 = ((dep1*w1)*(1-dpd1) + dpd1*w2) * (1-noc1); then force 1 at s==0
        m3 = const.tile([128, RPP], f32)
        inv = const.tile([128, RPP], f32)
        tmp = const.tile([128, RPP], f32)
        mul = mybir.AluOpType.mult
        add = mybir.AluOpType.add
        nc.vector.tensor_scalar(m3[:], dep1, w1T[:], None, op0=mul)
        nc.vector.tensor_scalar(inv[:], dpd1, -1.0, 1.0, op0=mul, op1=add)
        nc.vector.tensor_tensor(m3[:], m3[:], inv[:], op=mul)
        nc.vector.tensor_scalar(tmp[:], dpd1, w2T[:], None, op0=mul)
        nc.vector.tensor_tensor(m3[:], m3[:], tmp[:], op=add)
        nc.vector.tensor_scalar(inv[:], noc1, -1.0, 1.0, op0=mul, op1=add)
        nc.vector.tensor_tensor(m3[:], m3[:], inv[:], op=mul)
        nc.vector.tensor_scalar(inv[:], p0m, -1.0, 1.0, op0=mul, op1=add)
        nc.vector.tensor_tensor(m3[:], m3[:], inv[:], op=mul)
        nc.vector.tensor_tensor(m3[:], m3[:], p0m, op=add)

        # --- streamed multiply -------------------------------------------
        xr = x.rearrange("(p d q) h -> d p (q h)", p=128, d=ND)
        yr = [y.rearrange("(p d q) h -> d p (q h)", p=128, d=ND) for y in ys]
        for d in range(ND):
            xt = xpool.tile([128, RPT * H], f32, name="xt")
            nc.sync.dma_start(out=xt[:], in_=xr[d])
            yt = [ypool.tile([128, RPT * H], f32, name=f"y{i}t") for i in (1, 2, 3)]
            for g in range(RPT):
                r = d * RPT + g
                blk = slice(g * H, (g + 1) * H)
                nc.vector.tensor_scalar(
                    yt[0][:, blk], xt[:, blk], m1[:, r : r + 1], None, op0=mul
                )
                nc.vector.tensor_scalar(
                    yt[1][:, blk], xt[:, blk], m2[:, r : r + 1], None, op0=mul
                )
                nc.scalar.activation(
                    yt[2][:, blk], xt[:, blk],
                    mybir.ActivationFunctionType.Copy,
                    scale=m3[:, r : r + 1],
                )
            for i in range(3):
                nc.sync.dma_start(out=yr[i][d], in_=yt[i][:])

    _split_multiwaits(nc)
    return nc


def _prep_inputs(bert_local_out, depend, depended, no_connect,
                 depend_weight, depended_weight):
    x = np.ascontiguousarray(np.asarray(bert_local_out, dtype=np.float32))
    idx = {
        "dep": np.asarray(depend, dtype=np.int64),
        "dpd": np.asarray(depended, dtype=np.int64),
        "noc": np.asarray(no_connect, dtype=np.int64),
    }
    w1 = np.asarray(depend_weight, dtype=np.float32)
    w2 = np.asarray(depended_weight, dtype=np.float32)

    p0q = np.full(128, 9999.0, dtype=np.float32)
    p0r = np.full(128, 9999.0, dtype=np.float32)
    p0q[:BL] = 8 * np.arange(BL)
    p0r[:BL] = 0.0

    pidx = np.arange(128) // (128 // BL)  # sample owning each partition

    in_maps = []
    for c in range(N_CORES):
        sl = slice(c * BL, (c + 1) * BL)
        m = {
            "x": x[sl].reshape(ROWS, H),
            "p0q": p0q,
            "p0r": p0r,
            "w1p": w1[sl][pidx].astype(np.float32),
            "w2p": w2[sl][pidx].astype(np.float32),
        }
        boff = np.arange(BL, dtype=np.int64)[:, None] * S  # b*256
        for n, arr in idx.items():
            g = (arr[sl] + boff + 1).reshape(-1)  # global position + 1
            m[f"q_{n}"] = (g // RPP).astype(np.float32)
            m[f"r_{n}"] = (g % RPP).astype(np.float32)
        in_maps.append(m)
    return in_maps


def kernel(bert_local_out, depend, depended, no_connect,
           depend_weight, depended_weight):
    from concourse.bass_utils import run_bass_kernel_spmd

    if "nc" not in _cache:
        _cache["nc"] = _build()
    nc = _cache["nc"]

    in_maps = _prep_inputs(bert_local_out, depend, depended, no_connect,
                           depend_weight, depended_weight)

    pdir = os.environ.get("KERNEL_PROFILE_DIR")
    ctx = contextlib.nullcontext()
    if pdir:
        import concourse.bass2jax as b2j
        from trn_agent_boot.trn_boot import _ntff_profile_via_ctypes

        if not getattr(b2j, "_neff_capture_patched", False):
            orig = b2j.rename_neff_tensors_and_patch_header

            def patched(neff_path, mapping):
                data = orig(neff_path, mapping)
                cap = os.environ.get("KERNEL_PROFILE_DIR")
                if cap:
                    os.makedirs(cap, exist_ok=True)
                    with open(os.path.join(cap, "model.neff"), "wb") as f:
                        f.write(data)
                return data

            b2j.rename_neff_tensors_and_patch_header = patched
            b2j._neff_capture_patched = True
        os.makedirs(pdir, exist_ok=True)
        hookf = _ntff_profile_via_ctypes("/opt/axon/libaxon_pjrt.so")
        if hookf is not None:
            ctx = hookf(pdir, [0])

    with ctx:
        res = run_bass_kernel_spmd(nc, in_maps, list(range(N_CORES)))

    outs = []
    for name in ("y1", "y2", "y3"):
        full = np.empty((B, S, H), dtype=np.float32)
        for c in range(N_CORES):
            full[c * BL : (c + 1) * BL] = res.results[c][name].reshape(BL, S, H)
        outs.append(full)
    return tuple(outs)
